# revision 19
# baseline (speedup 1.0000x reference)
"""Trainium2 Bass kernel for DiagonalMultiplySum.

out[b, o, s] = sum_i input[b, i, s] * diagonal[o, i, s]

Shapes (hardcoded): input (64, 256, 4096) f32, diagonal (256, 256, 4096) f32,
output (64, 256, 4096) f32.

Strategy: shard the size axis across 8 NeuronCores (512 positions per core).
Each position s is an independent matmul out[:, :, s] = diag[:, :, s] @ in[:, :, s]^T
with contraction over i (256 -> 2 chunks of 128 on the PE partition dim).
Per position: diagonal is the stationary operand [K=128 i, M=128 o] (2 o-blocks),
input is the moving operand [K=128 i, N=64 b]; i-chunks accumulate in PSUM.
PSUM [o, b] tiles (8 positions per 2KB bank) are copied by DVE into an SBUF
staging tile and DMA'd out.  s-windows of W=32 positions, double buffered.
"""

import os
import sys

for _p in ("/opt/trn_rl_repo",):
    if _p not in sys.path and os.path.isdir(_p):
        sys.path.insert(0, _p)

import numpy as np

BATCH = 64
OUT_C = 256
IN_C = 256
SIZE = 4096
N_CORES = 8
S = SIZE // N_CORES  # 512 positions per core
P = 128

W = int(os.environ.get("DMS_W", "32"))  # positions per window
NW = S // W

_NC_CACHE = {}


def _build_nc():
    import concourse.bass as bass
    import concourse.mybir as mybir
    import concourse.tile as tile
    from contextlib import ExitStack

    fp32 = mybir.dt.float32
    nc = bass.Bass(trn_type="TRN2")

    inp = nc.dram_tensor("input", [BATCH, IN_C, S], fp32, kind="ExternalInput")
    dg = nc.dram_tensor("diagonal", [OUT_C, IN_C, S], fp32, kind="ExternalInput")
    out = nc.dram_tensor("output", [BATCH, OUT_C, S], fp32, kind="ExternalOutput")

    # DRAM access patterns.
    # input  (b, i, s) -> [p=i%128, (b ic), s]   ic = i//128
    in_src = inp.rearrange("b (ic p) s -> p (b ic) s", p=P)  # [128, 128, S]
    # diagonal (o, i, s) -> [p=i%128, (o ic), s]
    dg_src = dg.rearrange("o (ic p) s -> p (o ic) s", p=P)  # [128, 512, S]
    # output (b, o, s) -> [p=o%128, (b ob), s]   ob = o//128
    out_dst = out.rearrange("b (ob p) s -> p (b ob) s", p=P)  # [128, 128, S]

    with tile.TileContext(nc) as tc, ExitStack() as ctx:
        in_pool = ctx.enter_context(tc.tile_pool(name="inp", bufs=2))
        dg_pool = ctx.enter_context(tc.tile_pool(name="dgp", bufs=2))
        out_pool = ctx.enter_context(tc.tile_pool(name="outp", bufs=2))
        ps_pool = ctx.enter_context(tc.tile_pool(name="psp", bufs=7, space="PSUM"))
        dps_pool = ctx.enter_context(tc.tile_pool(name="dpsp", bufs=1, space="PSUM"))

        # Scratch PSUM bank for "wait absorber" dummy matmuls.  The walrus
        # codegen allows only ONE sync-wait per Matmult instruction, so each
        # window starts with two tiny matmuls that each absorb one DMA-completion
        # wait (dg and in); the real matmuls then carry at most one wait each.
        dps = dps_pool.tile([P, 8], fp32, name="dps")
        scratch_pool = ctx.enter_context(tc.tile_pool(name="scrp", bufs=1))
        scratch = scratch_pool.tile([1, 1], fp32, name="scratch")
        nc.vector.memset(scratch, 0.0)

        for w in range(NW):
            s0 = w * W

            # ---- loads ----
            # Split the big diag load across both HWDGE rings (SP + ACT);
            # descriptor generation is a bottleneck at 128B runs and the two
            # rings generate in parallel (~17% faster in microbench).
            in_t = in_pool.tile([P, 128 * W], fp32, name="in_t")
            in_t3 = in_t.rearrange("p (q s) -> p q s", s=W)
            nc.sync.dma_start(out=in_t3, in_=in_src[:, :, s0 : s0 + W])

            dg_t = dg_pool.tile([P, 512 * W], fp32, name="dg_t")
            dg_t3 = dg_t.rearrange("p (q s) -> p q s", s=W)
            nc.sync.dma_start(out=dg_t3[:, 0:256, :], in_=dg_src[:, 0:256, s0 : s0 + W])
            nc.scalar.dma_start(
                out=dg_t3[:, 256:512, :], in_=dg_src[:, 256:512, s0 : s0 + W]
            )

            # views for compute
            # in_t free layout (b, ic, s): [p, ic, b, s] strides (W, 2W, 1)
            in_t4 = in_t.rearrange("p (b ic s) -> p ic b s", ic=2, s=W)
            # dg_t free layout (ob, o, ic, s): [p, ob, ic, o, s]
            dg_t5 = dg_t.rearrange("p (ob o ic s) -> p ob ic o s", ob=2, ic=2, s=W)

            out_t = out_pool.tile([P, 128 * W], fp32, name="out_t")
            # out_t free layout (b, ob, s): [p, ob, b, s] strides (W, 2W, 1)
            out_t4 = out_t.rearrange("p (b ob s) -> p ob b s", ob=2, s=W)
            # DVE wait absorber: first touch of the recycled out_t slot carries
            # the WAR wait on the out-DMA of two windows ago, so the real PSUM
            # drain copies keep a single (PE) wait.
            nc.vector.tensor_copy(out_t[0:1, 0:1], scratch[0:1, 0:1])

            # ---- wait absorbers (see dps comment above) ----
            nc.tensor.matmul(
                dps[0:64, 0:1], dg_t5[:, 0, 0, 0:64, 0], dg_t5[:, 0, 0, 0:1, 0],
                start=True, stop=True,
            )
            nc.tensor.matmul(
                dps[0:64, 2:3], dg_t5[:, 1, 0, 0:64, 0], dg_t5[:, 1, 0, 0:1, 0],
                start=True, stop=True,
            )
            nc.tensor.matmul(
                dps[0:64, 1:2], in_t4[:, 0, :, 0], in_t4[:, 0, 0:1, 0],
                start=True, stop=True,
            )

            # ---- compute ----
            for ob in range(2):
                for s8 in range(W // 8):
                    ps = ps_pool.tile([P, 512], fp32, name="ps")
                    ps3 = ps.rearrange("p (q b) -> p q b", q=8)
                    for s_ib in range(8):
                        s_loc = s8 * 8 + s_ib
                        for ic in range(2):
                            nc.tensor.matmul(
                                ps3[:, s_ib, :],
                                dg_t5[:, ob, ic, :, s_loc],
                                in_t4[:, ic, :, s_loc],
                                start=(ic == 0),
                                stop=(ic == 1),
                            )
                    # drain bank: psum (s_ib, b) -> out_t (b, ob, s)
                    nc.vector.tensor_copy(
                        out_t4[:, ob, :, s8 * 8 : s8 * 8 + 8].transpose((0, 2, 1)),
                        ps3,
                    )

            # ---- store ----
            nc.scalar.dma_start(out=out_dst[:, :, s0 : s0 + W], in_=out_t3_view(out_t, W))

    _split_multi_waits(nc)
    return nc


def _split_multi_waits(nc):
    """Walrus codegen supports only ONE sync-wait per instruction.

    Tile emits multiple waits on some instructions; hoist all but the last
    onto same-engine NoOp instructions inserted immediately before the
    offender.  Per-engine in-order issue makes this exactly equivalent.
    """
    import concourse.mybir as mybir

    for f in nc.m.functions:
        for blk in f.blocks:
            new_list = []
            changed = False
            for inst in blk.instructions:
                si = inst.sync_info
                waits = list(si.on_wait) if si and si.on_wait else []
                if len(waits) > 1:
                    for w in waits[:-1]:
                        nop = mybir.InstNoOp(
                            name=nc.get_next_instruction_name(),
                            engine=inst.engine,
                            ins=[],
                            outs=[],
                            sync_info=mybir.SyncInfo(on_wait=[w], on_update=[]),
                        )
                        nc.register_instruction(nop)
                        new_list.append(nop)
                    si.on_wait = [waits[-1]]
                    changed = True
                new_list.append(inst)
            if changed:
                blk.instructions = new_list


def out_t3_view(out_t, w):
    return out_t.rearrange("p (q s) -> p q s", s=w)


def _get_nc():
    key = "nc"
    if key not in _NC_CACHE:
        _NC_CACHE[key] = _build_nc()
    return _NC_CACHE[key]


def kernel(**inputs):
    inp = np.asarray(inputs["input"], dtype=np.float32)
    dg = np.asarray(inputs["diagonal"], dtype=np.float32)
    assert inp.shape == (BATCH, IN_C, SIZE), inp.shape
    assert dg.shape == (OUT_C, IN_C, SIZE), dg.shape

    from concourse.bass_utils import run_bass_kernel_spmd

    nc = _get_nc()
    in_maps = []
    for c in range(N_CORES):
        sl = slice(c * S, (c + 1) * S)
        in_maps.append(
            {
                "input": np.ascontiguousarray(inp[:, :, sl]),
                "diagonal": np.ascontiguousarray(dg[:, :, sl]),
            }
        )
    res = run_bass_kernel_spmd(nc, in_maps, list(range(N_CORES)))
    out = np.concatenate([res.results[c]["output"] for c in range(N_CORES)], axis=2)
    return out


# revision 21
# speedup vs baseline: 1.3705x; 1.3705x over previous
"""Trainium2 Bass kernel for DiagonalMultiplySum.

out[b, o, s] = sum_i input[b, i, s] * diagonal[o, i, s]

Shapes (hardcoded): input (64, 256, 4096) f32, diagonal (256, 256, 4096) f32,
output (64, 256, 4096) f32.

Strategy: shard the size axis across 8 NeuronCores (512 positions per core).
Each position s is an independent matmul out[:, :, s] = diag[:, :, s] @ in[:, :, s]^T
with contraction over i (256 -> 2 chunks of 128 on the PE partition dim).
Per position: diagonal is the stationary operand [K=128 i, M=128 o] (2 o-blocks),
input is the moving operand [K=128 i, N=64 b]; i-chunks accumulate in PSUM.
PSUM [o, b] tiles (8 positions per 2KB bank) are copied by DVE into an SBUF
staging tile and DMA'd out.  s-windows of W=32 positions, double buffered.
"""

import os
import sys

for _p in ("/opt/trn_rl_repo",):
    if _p not in sys.path and os.path.isdir(_p):
        sys.path.insert(0, _p)

import numpy as np

BATCH = 64
OUT_C = 256
IN_C = 256
SIZE = 4096
N_CORES = 8
S = SIZE // N_CORES  # 512 positions per core
P = 128

W = int(os.environ.get("DMS_W", "32"))  # positions per window
NW = S // W

_NC_CACHE = {}


def _build_nc():
    import concourse.bass as bass
    import concourse.mybir as mybir
    import concourse.tile as tile
    from contextlib import ExitStack

    fp32 = mybir.dt.float32
    nc = bass.Bass(trn_type="TRN2")

    inp = nc.dram_tensor("input", [BATCH, IN_C, S], fp32, kind="ExternalInput")
    dg = nc.dram_tensor("diagonal", [OUT_C, IN_C, S], fp32, kind="ExternalInput")
    out = nc.dram_tensor("output", [BATCH, OUT_C, S], fp32, kind="ExternalOutput")

    # DRAM access patterns.
    # input  (b, i, s) -> [p=i%128, (b ic), s]   ic = i//128
    in_src = inp.rearrange("b (ic p) s -> p (b ic) s", p=P)  # [128, 128, S]
    # diagonal (o, i, s) -> [p=i%128, (o ic), s]
    dg_src = dg.rearrange("o (ic p) s -> p (o ic) s", p=P)  # [128, 512, S]
    # output (b, o, s) -> [p=o%128, (b ob), s]   ob = o//128
    out_dst = out.rearrange("b (ob p) s -> p (b ob) s", p=P)  # [128, 128, S]

    with tile.TileContext(nc) as tc, ExitStack() as ctx:
        in_pool = ctx.enter_context(tc.tile_pool(name="inp", bufs=2))
        dg_pool = ctx.enter_context(tc.tile_pool(name="dgp", bufs=2))
        out_pool = ctx.enter_context(tc.tile_pool(name="outp", bufs=2))
        ps_pool = ctx.enter_context(tc.tile_pool(name="psp", bufs=7, space="PSUM"))
        dps_pool = ctx.enter_context(tc.tile_pool(name="dpsp", bufs=1, space="PSUM"))

        # Scratch PSUM bank for "wait absorber" dummy matmuls.  The walrus
        # codegen allows only ONE sync-wait per Matmult instruction, so each
        # window starts with two tiny matmuls that each absorb one DMA-completion
        # wait (dg and in); the real matmuls then carry at most one wait each.
        dps = dps_pool.tile([P, 8], fp32, name="dps")
        scratch_pool = ctx.enter_context(tc.tile_pool(name="scrp", bufs=1))
        scratch = scratch_pool.tile([1, 1], fp32, name="scratch")
        nc.vector.memset(scratch, 0.0)

        for w in range(NW):
            s0 = w * W

            # ---- loads ----
            in_t = in_pool.tile([P, 128 * W], fp32, name="in_t")
            in_t3 = in_t.rearrange("p (q s) -> p q s", s=W)
            nc.scalar.dma_start(out=in_t3, in_=in_src[:, :, s0 : s0 + W])

            dg_t = dg_pool.tile([P, 512 * W], fp32, name="dg_t")
            dg_t3 = dg_t.rearrange("p (q s) -> p q s", s=W)
            nc.sync.dma_start(out=dg_t3, in_=dg_src[:, :, s0 : s0 + W])

            # views for compute
            # in_t free layout (b, ic, s): [p, ic, b, s] strides (W, 2W, 1)
            in_t4 = in_t.rearrange("p (b ic s) -> p ic b s", ic=2, s=W)
            # dg_t free layout (ob, o, ic, s): [p, ob, ic, o, s]
            dg_t5 = dg_t.rearrange("p (ob o ic s) -> p ob ic o s", ob=2, ic=2, s=W)

            out_t = out_pool.tile([P, 128 * W], fp32, name="out_t")
            # out_t free layout (b, ob, s): [p, ob, b, s] strides (W, 2W, 1)
            out_t4 = out_t.rearrange("p (b ob s) -> p ob b s", ob=2, s=W)
            # DVE wait absorber: first touch of the recycled out_t slot carries
            # the WAR wait on the out-DMA of two windows ago, so the real PSUM
            # drain copies keep a single (PE) wait.
            nc.vector.tensor_copy(out_t[0:1, 0:1], scratch[0:1, 0:1])

            # ---- wait absorbers (see dps comment above) ----
            nc.tensor.matmul(
                dps[0:64, 0:1], dg_t5[:, 0, 0, 0:64, 0], dg_t5[:, 0, 0, 0:1, 0],
                start=True, stop=True,
            )
            nc.tensor.matmul(
                dps[0:64, 1:2], in_t4[:, 0, :, 0], in_t4[:, 0, 0:1, 0],
                start=True, stop=True,
            )

            # ---- compute ----
            for ob in range(2):
                for s8 in range(W // 8):
                    ps = ps_pool.tile([P, 512], fp32, name="ps")
                    ps3 = ps.rearrange("p (q b) -> p q b", q=8)
                    for s_ib in range(8):
                        s_loc = s8 * 8 + s_ib
                        for ic in range(2):
                            nc.tensor.matmul(
                                ps3[:, s_ib, :],
                                dg_t5[:, ob, ic, :, s_loc],
                                in_t4[:, ic, :, s_loc],
                                start=(ic == 0),
                                stop=(ic == 1),
                            )
                    # drain bank: psum (s_ib, b) -> out_t (b, ob, s)
                    nc.vector.tensor_copy(
                        out_t4[:, ob, :, s8 * 8 : s8 * 8 + 8].transpose((0, 2, 1)),
                        ps3,
                    )

            # ---- store ----
            nc.scalar.dma_start(out=out_dst[:, :, s0 : s0 + W], in_=out_t3_view(out_t, W))

    _split_multi_waits(nc)
    return nc


def _split_multi_waits(nc):
    """Walrus codegen supports only ONE sync-wait per instruction.

    Tile emits multiple waits on some instructions; hoist all but the last
    onto same-engine NoOp instructions inserted immediately before the
    offender.  Per-engine in-order issue makes this exactly equivalent.
    """
    import concourse.mybir as mybir

    for f in nc.m.functions:
        for blk in f.blocks:
            new_list = []
            changed = False
            for inst in blk.instructions:
                si = inst.sync_info
                waits = list(si.on_wait) if si and si.on_wait else []
                if len(waits) > 1:
                    for w in waits[:-1]:
                        nop = mybir.InstNoOp(
                            name=nc.get_next_instruction_name(),
                            engine=inst.engine,
                            ins=[],
                            outs=[],
                            sync_info=mybir.SyncInfo(on_wait=[w], on_update=[]),
                        )
                        nc.register_instruction(nop)
                        new_list.append(nop)
                    si.on_wait = [waits[-1]]
                    changed = True
                new_list.append(inst)
            if changed:
                blk.instructions = new_list


def out_t3_view(out_t, w):
    return out_t.rearrange("p (q s) -> p q s", s=w)


def _get_nc():
    key = "nc"
    if key not in _NC_CACHE:
        _NC_CACHE[key] = _build_nc()
    return _NC_CACHE[key]


def kernel(**inputs):
    inp = np.asarray(inputs["input"], dtype=np.float32)
    dg = np.asarray(inputs["diagonal"], dtype=np.float32)
    assert inp.shape == (BATCH, IN_C, SIZE), inp.shape
    assert dg.shape == (OUT_C, IN_C, SIZE), dg.shape

    from concourse.bass_utils import run_bass_kernel_spmd

    nc = _get_nc()
    in_maps = []
    for c in range(N_CORES):
        sl = slice(c * S, (c + 1) * S)
        in_maps.append(
            {
                "input": np.ascontiguousarray(inp[:, :, sl]),
                "diagonal": np.ascontiguousarray(dg[:, :, sl]),
            }
        )
    res = run_bass_kernel_spmd(nc, in_maps, list(range(N_CORES)))
    out = np.concatenate([res.results[c]["output"] for c in range(N_CORES)], axis=2)
    return out


# revision 22
# speedup vs baseline: 1.3959x; 1.0185x over previous
"""Trainium2 Bass kernel for DiagonalMultiplySum.

out[b, o, s] = sum_i input[b, i, s] * diagonal[o, i, s]

Shapes (hardcoded): input (64, 256, 4096) f32, diagonal (256, 256, 4096) f32,
output (64, 256, 4096) f32.

Strategy: shard the size axis across 8 NeuronCores (512 positions per core).
Each position s is an independent matmul out[:, :, s] = diag[:, :, s] @ in[:, :, s]^T
with contraction over i (256 -> 2 chunks of 128 on the PE partition dim).
Per position: diagonal is the stationary operand [K=128 i, M=128 o] (2 o-blocks),
input is the moving operand [K=128 i, N=64 b]; i-chunks accumulate in PSUM.
PSUM [o, b] tiles (8 positions per 2KB bank) are copied by DVE into an SBUF
staging tile and DMA'd out.  s-windows of W=32 positions, double buffered.
"""

import os
import sys

for _p in ("/opt/trn_rl_repo",):
    if _p not in sys.path and os.path.isdir(_p):
        sys.path.insert(0, _p)

import numpy as np

BATCH = 64
OUT_C = 256
IN_C = 256
SIZE = 4096
N_CORES = 8
S = SIZE // N_CORES  # 512 positions per core
P = 128

W = int(os.environ.get("DMS_W", "32"))  # positions per window
NW = S // W

_NC_CACHE = {}


def _build_nc():
    import concourse.bass as bass
    import concourse.mybir as mybir
    import concourse.tile as tile
    from contextlib import ExitStack

    fp32 = mybir.dt.float32
    nc = bass.Bass(trn_type="TRN2")

    inp = nc.dram_tensor("input", [BATCH, IN_C, S], fp32, kind="ExternalInput")
    dg = nc.dram_tensor("diagonal", [OUT_C, IN_C, S], fp32, kind="ExternalInput")
    out = nc.dram_tensor("output", [BATCH, OUT_C, S], fp32, kind="ExternalOutput")

    # DRAM access patterns.
    # input  (b, i, s) -> [p=i%128, (b ic), s]   ic = i//128
    in_src = inp.rearrange("b (ic p) s -> p (b ic) s", p=P)  # [128, 128, S]
    # diagonal (o, i, s) -> [p=i%128, (o ic), s]
    dg_src = dg.rearrange("o (ic p) s -> p (o ic) s", p=P)  # [128, 512, S]
    # output (b, o, s) -> [p=o%128, (b ob), s]   ob = o//128
    out_dst = out.rearrange("b (ob p) s -> p (b ob) s", p=P)  # [128, 128, S]

    with tile.TileContext(nc) as tc, ExitStack() as ctx:
        in_pool = ctx.enter_context(tc.tile_pool(name="inp", bufs=2))
        dg_pool = ctx.enter_context(tc.tile_pool(name="dgp", bufs=2))
        out_pool = ctx.enter_context(tc.tile_pool(name="outp", bufs=2))
        ps_pool = ctx.enter_context(tc.tile_pool(name="psp", bufs=7, space="PSUM"))
        dps_pool = ctx.enter_context(tc.tile_pool(name="dpsp", bufs=1, space="PSUM"))

        # Scratch PSUM bank for "wait absorber" dummy matmuls: walrus allows
        # only ONE sync-wait per Matmult, so each window/pass starts with tiny
        # matmuls that absorb the DMA-completion waits (see _split_multi_waits
        # for the general fallback).
        dps = dps_pool.tile([P, 8], fp32, name="dps")

        # s-windows of up to WS positions; each window is processed in two
        # o-half passes so only half the diagonal (128 o x 2 ic) is resident
        # at a time -> 1.5x longer DMA runs than the all-o layout.
        # Compute: input is the STATIONARY operand [K=128 i, M=64 b] (cheap
        # fp32 weight loads), diag o-half is MOVING [K=128, N=128 o];
        # PSUM out = [b:64, o:128] per position, 4 positions per bank.
        WSIZES = [48] * 10 + [32]
        assert sum(WSIZES) == S
        s0 = 0
        for w, WS in enumerate(WSIZES):
            in_t = in_pool.tile([P, 128 * WS], fp32, name="in_t", tag="in_t")
            in_t3 = in_t.rearrange("p (q s) -> p q s", s=WS)
            nc.scalar.dma_start(out=in_t3, in_=in_src[:, :, s0 : s0 + WS])
            in_t4 = in_t.rearrange("p (b ic s) -> p ic b s", ic=2, s=WS)

            for half in range(2):
                dg_t = dg_pool.tile([P, 256 * WS], fp32, name="dg_t", tag="dg_t")
                dg_t3 = dg_t.rearrange("p (q s) -> p q s", s=WS)
                nc.sync.dma_start(
                    out=dg_t3,
                    in_=dg_src[:, half * 256 : (half + 1) * 256, s0 : s0 + WS],
                )
                # free layout (o', ic, s): moving rhs [p, 128 o] stride 2*WS
                dg_t4 = dg_t.rearrange("p (o ic s) -> p ic o s", ic=2, s=WS)

                out_t = out_pool.tile([64, 128 * WS], fp32, name="out_t", tag="out_t")
                out_t3 = out_t.rearrange("p (o s) -> p o s", s=WS)

                # wait absorbers
                nc.tensor.matmul(
                    dps[0:64, half : half + 1],
                    dg_t4[:, 0, 0:64, 0],
                    dg_t4[:, 0, 0:1, 0],
                    start=True, stop=True,
                )
                if half == 0:
                    nc.tensor.matmul(
                        dps[0:64, 2:3],
                        in_t4[:, 0, :, 0],
                        in_t4[:, 0, 0:1, 0],
                        start=True, stop=True,
                    )

                for g in range(WS // 4):
                    ps = ps_pool.tile([64, 512], fp32, name="ps", tag="ps")
                    ps3 = ps.rearrange("p (g o) -> p g o", g=4)
                    for j in range(4):
                        s_loc = g * 4 + j
                        for ic in range(2):
                            nc.tensor.matmul(
                                ps3[:, j, :],
                                in_t4[:, ic, :, s_loc],
                                dg_t4[:, ic, :, s_loc],
                                start=(ic == 0),
                                stop=(ic == 1),
                            )
                    # drain bank: psum (j, o) -> out_t (o, s)
                    nc.vector.tensor_copy(
                        out_t3[:, :, g * 4 : g * 4 + 4].transpose((0, 2, 1)),
                        ps3,
                    )

                nc.scalar.dma_start(
                    out=out[:, half * 128 : (half + 1) * 128, s0 : s0 + WS],
                    in_=out_t3,
                )
            s0 += WS

    _split_multi_waits(nc)
    return nc


def _split_multi_waits(nc):
    """Walrus codegen supports only ONE sync-wait per instruction.

    Tile emits multiple waits on some instructions; hoist all but the last
    onto same-engine NoOp instructions inserted immediately before the
    offender.  Per-engine in-order issue makes this exactly equivalent.
    """
    import concourse.mybir as mybir

    for f in nc.m.functions:
        for blk in f.blocks:
            new_list = []
            changed = False
            for inst in blk.instructions:
                si = inst.sync_info
                waits = list(si.on_wait) if si and si.on_wait else []
                if len(waits) > 1:
                    for w in waits[:-1]:
                        nop = mybir.InstNoOp(
                            name=nc.get_next_instruction_name(),
                            engine=inst.engine,
                            ins=[],
                            outs=[],
                            sync_info=mybir.SyncInfo(on_wait=[w], on_update=[]),
                        )
                        nc.register_instruction(nop)
                        new_list.append(nop)
                    si.on_wait = [waits[-1]]
                    changed = True
                new_list.append(inst)
            if changed:
                blk.instructions = new_list


def out_t3_view(out_t, w):
    return out_t.rearrange("p (q s) -> p q s", s=w)


def _get_nc():
    key = "nc"
    if key not in _NC_CACHE:
        _NC_CACHE[key] = _build_nc()
    return _NC_CACHE[key]


def kernel(**inputs):
    inp = np.asarray(inputs["input"], dtype=np.float32)
    dg = np.asarray(inputs["diagonal"], dtype=np.float32)
    assert inp.shape == (BATCH, IN_C, SIZE), inp.shape
    assert dg.shape == (OUT_C, IN_C, SIZE), dg.shape

    from concourse.bass_utils import run_bass_kernel_spmd

    nc = _get_nc()
    in_maps = []
    for c in range(N_CORES):
        sl = slice(c * S, (c + 1) * S)
        in_maps.append(
            {
                "input": np.ascontiguousarray(inp[:, :, sl]),
                "diagonal": np.ascontiguousarray(dg[:, :, sl]),
            }
        )
    res = run_bass_kernel_spmd(nc, in_maps, list(range(N_CORES)))
    out = np.concatenate([res.results[c]["output"] for c in range(N_CORES)], axis=2)
    return out
